# revision 13
# baseline (speedup 1.0000x reference)
"""DeepSeek-style local-window sparse attention on 8 TRN2 NeuronCores.

Problem: B=2, N=2048, D=768, H=12 heads x d=64, local window |q-k| <= 64,
out = softmax(mask(q k^T / 8)) v  projected by Wo.

Sharding (no on-device collectives):
  core c in 0..7 -> batch b = c//4, head group g = c%4 (heads 3g..3g+2).
  Each core computes its 3 heads' q/k/v projections over the full
  sequence, the banded attention, and a PARTIAL output projection
  (its 192 rows of Wo). The host sums the 4 partials per batch.

Device layout choices:
  - x is fed pre-transposed (xT [768, 2048]) so projections need no
    on-device transpose: q^T/k^T come out d-major (x^T as moving
    operand), v comes out token-major (x^T as stationary operand).
  - scores are computed transposed (S^T[tk, tq]) so the attn@v matmul
    takes exp(S^T) directly as the stationary operand; the softmax
    denominator falls out of the same matmul via a ones-column
    appended to v; normalization is a per-partition scalar multiply.
  - softmax skips the running-max: scores*scale here are ~N(0, 0.31),
    so exp never overflows (verified against the reference).
  - bk shifts scores per-query-row only (softmax-invariant) but is
    applied anyway with bq via the free per-partition bias slot of the
    PSUM->SBUF copy. bv and bo pass through the softmax/projection
    linearly and are added on the host: out += bv @ Wo + bo.
  - key blocks per 128-query block i: the three aligned 128-key blocks
    i-1, i, i+1 with fixed triangular/band 0/1 masks multiplied into
    exp(S^T); edge blocks just skip the missing key block.

Matmul operands are bf16 (f32 PSUM accumulation); partial outputs are
returned f32 and reduced on the host.
"""

import numpy as np
import ml_dtypes

import concourse.bass as bass
import concourse.tile as tile
from concourse import mybir
from concourse.bass_utils import run_bass_kernel_spmd
from concourse.vector_clock import ScopedClock

BF16 = mybir.dt.bfloat16
F32 = mybir.dt.float32
AF = mybir.ActivationFunctionType

B, N, D = 2, 2048, 768
H, DH = 12, 64
HPC = 3              # heads per core
GC = HPC * DH        # 192 output columns per core
NB = N // 128        # 16 query blocks
KC = D // 128        # 6 contraction chunks
SCALE = 0.125        # 1/sqrt(64)
NCORES = 8


def _split_multiwaits(nc):
    """Hoist extra semaphore waits onto standalone EventSemaphore ops.

    The walrus build in this container rejects any engine instruction
    carrying more than one sync wait ("Too many sync wait commands");
    Tile's semaphore assignment freely attaches several. An engine
    sequencer executes instructions in order, so waiting on A via a
    standalone EventSemaphore then on B via the instruction itself is
    equivalent to one instruction waiting on both. DMA transfers are
    left untouched (their waits live in DGE descriptors).
    """
    n = 0
    for fn in nc.m.functions:
        for bb in fn.blocks:
            out = []
            for inst in bb.instructions:
                si = inst.sync_info
                if si is not None and len(si.on_wait) > 1:
                    extras = list(si.on_wait[:-1])
                    si.on_wait = [si.on_wait[-1]]
                    for w in extras:
                        es = mybir.InstEventSemaphore(
                            name=f"splitw_{n}", ins=[], outs=[]
                        )
                        n += 1
                        es.engine = inst.engine
                        es.sync_info = mybir.SyncInfo(on_wait=[w], on_update=[])
                        nc.register_instruction(es)
                        out.append(es)
                out.append(inst)
            bb.instructions = out


def _emit(nc, tc, d):
    from contextlib import ExitStack

    with ExitStack() as ctx:
        const = ctx.enter_context(tc.tile_pool(name="const", bufs=1))
        persist = ctx.enter_context(tc.tile_pool(name="persist", bufs=1))
        ps_big = ctx.enter_context(tc.tile_pool(name="ps_big", bufs=2, space="PSUM"))
        ps_s = ctx.enter_context(tc.tile_pool(name="ps_s", bufs=3, space="PSUM"))
        ps_sm = ctx.enter_context(tc.tile_pool(name="ps_sm", bufs=3, space="PSUM"))
        e_pool = ctx.enter_context(tc.tile_pool(name="e_pool", bufs=4))
        ao_pool = ctx.enter_context(tc.tile_pool(name="ao_pool", bufs=2))
        o_pool = ctx.enter_context(tc.tile_pool(name="o_pool", bufs=3))
        zr_pool = ctx.enter_context(tc.tile_pool(name="zr_pool", bufs=4))

        # ---- load constants / weights / activations ----
        xts = []
        wqk_sb = []
        wv_sb = []
        for k in range(KC):
            xt = const.tile([128, N], BF16, name=f"xts{k}", tag=f"xts{k}")
            nc.sync.dma_start(xt[:], d["xT"][128 * k : 128 * (k + 1), :])
            xts.append(xt)
            wq = const.tile([128, 512], BF16, name=f"wqk{k}", tag=f"wqk{k}")
            nc.sync.dma_start(wq[:], d["wqk"][128 * k : 128 * (k + 1), :])
            wqk_sb.append(wq)
            wv = const.tile([128, GC], BF16, name=f"wv{k}", tag=f"wv{k}")
            nc.sync.dma_start(wv[:], d["wv"][128 * k : 128 * (k + 1), :])
            wv_sb.append(wv)
        wo0_sb = const.tile([128, D], BF16, name="wo0", tag="wo0")
        nc.sync.dma_start(wo0_sb[:], d["wo0"][:, :])
        wo1_sb = const.tile([64, D], BF16, name="wo1", tag="wo1")
        nc.sync.dma_start(wo1_sb[:], d["wo1"][:, :])
        bqk_sb = const.tile([128, 4], F32, name="bqk", tag="bqk")
        nc.sync.dma_start(bqk_sb[:], d["bqk"][:, :])
        masks_sb = const.tile([128, 384], BF16, name="masks", tag="masks")
        nc.sync.dma_start(masks_sb[:], d["masks"][:, :])
        ident_sb = const.tile([128, 128], BF16, name="ident", tag="ident")
        nc.sync.dma_start(ident_sb[:], d["ident"][:, :])

        # ---- persistent intermediates ----
        # packed [q0|q1], [q2|pad], [k0|k1], [k2|pad] d-major, 4 chunks of
        # 128 rows x N cols (padding keeps q_h and k_h at the same
        # partition offset — matmul requires matching base partitions)
        qk_sb = [
            persist.tile([128, N], BF16, name=f"qk{m}", tag=f"qk{m}") for m in range(4)
        ]
        # v token-major with a ones column per head: per 128-token chunk the
        # 195 cols are [v_h0(64) | 1 | v_h1(64) | 1 | v_h2(64) | 1]
        v3a = persist.tile([128, NB * 195], BF16, name="v3a", tag="v3a")
        ones_ap = v3a[:].rearrange("p (c g s) -> p c g s", c=NB, g=HPC, s=65)[
            :, :, :, 64:65
        ]
        nc.vector.memset(ones_ap, 1.0)
        # attention output, head-major (A0: packed rows 0..127, A1: 128..191)
        a0_sb = persist.tile([128, N], BF16, name="a0", tag="a0")
        a1_sb = persist.tile([64, N], BF16, name="a1", tag="a1")

        # ---- q/k projections: qk^T[m] = wqk[:, m-chunk]^T @ x^T ----
        for m in range(4):
            for n in range(N // 512):
                ps = ps_big.tile([128, 512], F32, name=f"psqk{m}_{n}", tag="big")
                for k in range(KC):
                    nc.tensor.matmul(
                        ps[:],
                        wqk_sb[k][:, 128 * m : 128 * (m + 1)],
                        xts[k][:, 512 * n : 512 * (n + 1)],
                        start=(k == 0),
                        stop=(k == KC - 1),
                    )
                nc.scalar.activation(
                    qk_sb[m][:, 512 * n : 512 * (n + 1)],
                    ps[:],
                    AF.Identity,
                    bias=bqk_sb[:, m : m + 1],
                    scale=1.0,
                )

        # ---- v projection (token-major): v[i-chunk] = x^T[:, chunk]^T @ wv ----
        for i in range(NB):
            psv = ps_big.tile([128, GC], F32, name=f"psv{i}", tag="big")
            for k in range(KC):
                nc.tensor.matmul(
                    psv[:],
                    xts[k][:, 128 * i : 128 * (i + 1)],
                    wv_sb[k][:],
                    start=(k == 0),
                    stop=(k == KC - 1),
                )
            vout = v3a[:, 195 * i : 195 * i + 195].rearrange(
                "p (g s) -> p g s", g=HPC, s=65
            )[:, :, 0:64]
            vin = psv[:].rearrange("p (g s) -> p g s", g=HPC, s=DH)
            nc.vector.tensor_copy(vout, vin)

        # ---- banded attention ----
        for i in range(NB):
            c_first = 1 if i == 0 else 0
            c_last = 1 if i == NB - 1 else 2
            lo, hi = 128 * c_first, 128 * (c_last + 1)
            ao3 = ao_pool.tile([128, GC], BF16, name=f"ao{i}", tag="ao")
            for h in range(HPC):
                # S^T[tk, tq] for key blocks j = i-1+c
                pss = ps_s.tile([128, 384], F32, name=f"pss{i}_{h}", tag="s")
                mq, qo = (0, 64 * h) if h < 2 else (1, 0)
                mk, ko = (2, 64 * h) if h < 2 else (3, 0)
                for c in range(c_first, c_last + 1):
                    j = i - 1 + c
                    nc.tensor.matmul(
                        pss[:, 128 * c : 128 * (c + 1)],
                        qk_sb[mk][ko : ko + 64, 128 * j : 128 * (j + 1)],
                        qk_sb[mq][qo : qo + 64, 128 * i : 128 * (i + 1)],
                        start=True,
                        stop=True,
                    )
                e = e_pool.tile([128, 384], BF16, name=f"e{i}_{h}", tag="e")
                nc.scalar.activation(e[:, lo:hi], pss[:, lo:hi], AF.Exp, scale=SCALE)
                nc.vector.tensor_mul(e[:, lo:hi], e[:, lo:hi], masks_sb[:, lo:hi])
                # attn @ [v | 1]: accumulates out[tq, 0:64] and Z[tq] in col 64
                pso = ps_sm.tile([128, 65], F32, name=f"pso{i}_{h}", tag="sm")
                for c in range(c_first, c_last + 1):
                    j = i - 1 + c
                    nc.tensor.matmul(
                        pso[:],
                        e[:, 128 * c : 128 * (c + 1)],
                        v3a[:, 195 * j + 65 * h : 195 * j + 65 * h + 65],
                        start=(c == c_first),
                        stop=(c == c_last),
                    )
                zr = zr_pool.tile([128, 1], F32, name=f"zr{i}_{h}", tag="zr")
                nc.vector.reciprocal(zr[:], pso[:, 64:65])
                nc.vector.tensor_scalar_mul(
                    ao3[:, 64 * h : 64 * (h + 1)], pso[:, 0:64], zr[:]
                )
            # transpose [tq, 192] -> head-major [192, tq] for the out-proj
            pt0 = ps_sm.tile([128, 128], BF16, name=f"pt0_{i}", tag="sm")
            nc.tensor.transpose(pt0[:], ao3[:, 0:128], ident_sb[:])
            pt1 = ps_sm.tile([64, 128], BF16, name=f"pt1_{i}", tag="sm")
            nc.tensor.transpose(pt1[:], ao3[:, 128:192], ident_sb[:])
            nc.vector.tensor_copy(a0_sb[:, 128 * i : 128 * (i + 1)], pt0[:])
            nc.vector.tensor_copy(a1_sb[:, 128 * i : 128 * (i + 1)], pt1[:])

        # ---- partial out-projection: outT = wo_rows^T @ attn^T ----
        for n in range(N // 512):
            for dd in range(KC):
                psp = ps_big.tile([128, 512], F32, name=f"psp{n}_{dd}", tag="big")
                nc.tensor.matmul(
                    psp[:],
                    wo0_sb[:, 128 * dd : 128 * (dd + 1)],
                    a0_sb[:, 512 * n : 512 * (n + 1)],
                    start=True,
                    stop=False,
                )
                nc.tensor.matmul(
                    psp[:],
                    wo1_sb[:, 128 * dd : 128 * (dd + 1)],
                    a1_sb[:, 512 * n : 512 * (n + 1)],
                    start=False,
                    stop=True,
                )
                osb = o_pool.tile([128, 512], F32, name=f"os{n}_{dd}", tag="o")
                nc.scalar.copy(osb[:], psp[:])
                nc.sync.dma_start(
                    d["outT"][128 * dd : 128 * (dd + 1), 512 * n : 512 * (n + 1)],
                    osb[:],
                )


_CACHED_NC = None


def build_nc():
    global _CACHED_NC
    if _CACHED_NC is not None:
        return _CACHED_NC
    nc = bass.Bass("TRN2", target_bir_lowering=False, debug=False, num_devices=NCORES)
    d = {
        "xT": nc.dram_tensor("xT", [D, N], BF16, kind="ExternalInput").ap(),
        "wqk": nc.dram_tensor("wqk", [D, 512], BF16, kind="ExternalInput").ap(),
        "wv": nc.dram_tensor("wv", [D, GC], BF16, kind="ExternalInput").ap(),
        "wo0": nc.dram_tensor("wo0", [128, D], BF16, kind="ExternalInput").ap(),
        "wo1": nc.dram_tensor("wo1", [64, D], BF16, kind="ExternalInput").ap(),
        "bqk": nc.dram_tensor("bqk", [128, 4], F32, kind="ExternalInput").ap(),
        "masks": nc.dram_tensor("masks", [128, 384], BF16, kind="ExternalInput").ap(),
        "ident": nc.dram_tensor("ident", [128, 128], BF16, kind="ExternalInput").ap(),
        "outT": nc.dram_tensor("outT", [D, N], F32, kind="ExternalOutput").ap(),
    }
    with tile.TileContext(nc) as tc:
        _emit(nc, tc, d)
    _split_multiwaits(nc)
    _CACHED_NC = nc
    return nc


def _build_masks():
    p = np.arange(128)[:, None]
    f = np.arange(128)[None, :]
    m = np.zeros((128, 384), np.float32)
    m[:, 0:128] = (p - f >= 64).astype(np.float32)    # key block i-1
    m[:, 128:256] = (np.abs(p - f) <= 64).astype(np.float32)  # key block i
    m[:, 256:384] = (p - f <= -64).astype(np.float32)  # key block i+1
    return m.astype(ml_dtypes.bfloat16)


def make_in_maps(x, Wq, bq, Wk, bk, Wv, bv, Wo, bo):
    bf = ml_dtypes.bfloat16
    masks = _build_masks()
    ident = np.eye(128, dtype=bf)
    xT = [np.ascontiguousarray(x[b].T).astype(bf) for b in range(B)]
    in_maps = []
    pad64 = np.zeros((D, 64), np.float32)
    padb = np.zeros(64, np.float32)
    for c in range(NCORES):
        b, g = divmod(c, 4)
        s = slice(GC * g, GC * (g + 1))
        # chunks: [q0|q1], [q2|pad], [k0|k1], [k2|pad]
        wqk = np.concatenate(
            [Wq[:, s][:, :128], Wq[:, s][:, 128:], pad64,
             Wk[:, s][:, :128], Wk[:, s][:, 128:], pad64],
            axis=1,
        ).astype(bf)
        wv = np.ascontiguousarray(Wv[:, s]).astype(bf)
        wo = Wo[s, :]
        bqk = np.concatenate(
            [bq[s][:128], bq[s][128:], padb, bk[s][:128], bk[s][128:], padb]
        ).reshape(4, 128).T
        in_maps.append(
            {
                "xT": xT[b],
                "wqk": wqk,
                "wv": wv,
                "wo0": np.ascontiguousarray(wo[0:128, :]).astype(bf),
                "wo1": np.ascontiguousarray(wo[128:GC, :]).astype(bf),
                "bqk": np.ascontiguousarray(bqk, dtype=np.float32),
                "masks": masks,
                "ident": ident,
            }
        )
    return in_maps


def combine_outputs(partials, Wq, bq, Wk, bk, Wv, bv, Wo, bo):
    const = (bv.astype(np.float32) @ Wo.astype(np.float32) + bo).astype(np.float32)
    out = np.empty((B, N, D), np.float32)
    for b in range(B):
        acc = partials[4 * b].astype(np.float32).copy()
        for c in range(4 * b + 1, 4 * b + 4):
            acc += partials[c]
        out[b] = acc.T + const
    return out


def kernel(x, Wq, bq, Wk, bk, Wv, bv, Wo, bo, _trace=False, **run_kwargs):
    x = np.asarray(x, dtype=np.float32)
    args = [np.asarray(a, dtype=np.float32) for a in (Wq, bq, Wk, bk, Wv, bv, Wo, bo)]
    nc = build_nc()
    in_maps = make_in_maps(x, *args)
    res = run_bass_kernel_spmd(
        nc, in_maps, core_ids=list(range(NCORES)), trace=_trace, **run_kwargs
    )
    partials = [res.results[c]["outT"] for c in range(NCORES)]
    out = combine_outputs(partials, *args)
    if _trace:
        kernel.last_results = res
    return out


# revision 15
# speedup vs baseline: 1.0255x; 1.0255x over previous
"""DeepSeek-style local-window sparse attention on 8 TRN2 NeuronCores.

Problem: B=2, N=2048, D=768, H=12 heads x d=64, local window |q-k| <= 64,
out = softmax(mask(q k^T / 8)) v  projected by Wo.

Sharding (no on-device collectives):
  core c in 0..7 -> batch b = c//4, head group g = c%4 (heads 3g..3g+2).
  Each core computes its 3 heads' q/k/v projections over the full
  sequence, the banded attention, and a PARTIAL output projection
  (its 192 rows of Wo). The host sums the 4 partials per batch.

Device layout choices:
  - x is fed pre-transposed (xT [768, 2048]) so projections need no
    on-device transpose: q^T/k^T come out d-major (x^T as moving
    operand), v comes out token-major (x^T as stationary operand).
  - scores are computed transposed (S^T[tk, tq]) so the attn@v matmul
    takes exp(S^T) directly as the stationary operand; the softmax
    denominator falls out of the same matmul via a ones-column
    appended to v; normalization is a per-partition scalar multiply.
  - softmax skips the running-max: scores*scale here are ~N(0, 0.31),
    so exp never overflows (verified against the reference).
  - bk shifts scores per-query-row only (softmax-invariant) but is
    applied anyway with bq via the free per-partition bias slot of the
    PSUM->SBUF copy. bv and bo pass through the softmax/projection
    linearly and are added on the host: out += bv @ Wo + bo.
  - key blocks per 128-query block i: the three aligned 128-key blocks
    i-1, i, i+1 with fixed triangular/band 0/1 masks multiplied into
    exp(S^T); edge blocks just skip the missing key block.

Matmul operands are bf16 (f32 PSUM accumulation); partial outputs are
returned f32 and reduced on the host.
"""

import numpy as np
import ml_dtypes

import concourse.bass as bass
import concourse.tile as tile
from concourse import mybir
from concourse.bass_utils import run_bass_kernel_spmd
from concourse.vector_clock import ScopedClock

BF16 = mybir.dt.bfloat16
F32 = mybir.dt.float32
AF = mybir.ActivationFunctionType

B, N, D = 2, 2048, 768
H, DH = 12, 64
HPC = 3              # heads per core
GC = HPC * DH        # 192 output columns per core
NB = N // 128        # 16 query blocks
KC = D // 128        # 6 contraction chunks
SCALE = 0.125        # 1/sqrt(64)
NCORES = 8


def _split_multiwaits(nc):
    """Hoist extra semaphore waits onto standalone EventSemaphore ops.

    The walrus build in this container rejects any engine instruction
    carrying more than one sync wait ("Too many sync wait commands");
    Tile's semaphore assignment freely attaches several. An engine
    sequencer executes instructions in order, so waiting on A via a
    standalone EventSemaphore then on B via the instruction itself is
    equivalent to one instruction waiting on both. DMA transfers are
    left untouched (their waits live in DGE descriptors).
    """
    n = 0
    for fn in nc.m.functions:
        for bb in fn.blocks:
            out = []
            for inst in bb.instructions:
                si = inst.sync_info
                if si is not None and len(si.on_wait) > 1:
                    extras = list(si.on_wait[:-1])
                    si.on_wait = [si.on_wait[-1]]
                    for w in extras:
                        es = mybir.InstEventSemaphore(
                            name=f"splitw_{n}", ins=[], outs=[]
                        )
                        n += 1
                        es.engine = inst.engine
                        es.sync_info = mybir.SyncInfo(on_wait=[w], on_update=[])
                        nc.register_instruction(es)
                        out.append(es)
                out.append(inst)
            bb.instructions = out


def _emit(nc, tc, d):
    from contextlib import ExitStack

    with ExitStack() as ctx:
        const = ctx.enter_context(tc.tile_pool(name="const", bufs=1))
        persist = ctx.enter_context(tc.tile_pool(name="persist", bufs=1))
        ps_big = ctx.enter_context(tc.tile_pool(name="ps_big", bufs=2, space="PSUM"))
        ps_s = ctx.enter_context(tc.tile_pool(name="ps_s", bufs=3, space="PSUM"))
        ps_sm = ctx.enter_context(tc.tile_pool(name="ps_sm", bufs=3, space="PSUM"))
        e_pool = ctx.enter_context(tc.tile_pool(name="e_pool", bufs=10))
        ao_pool = ctx.enter_context(tc.tile_pool(name="ao_pool", bufs=3))
        o_pool = ctx.enter_context(tc.tile_pool(name="o_pool", bufs=3))
        zr_pool = ctx.enter_context(tc.tile_pool(name="zr_pool", bufs=4))

        # ---- load constants / weights / activations ----
        xts = []
        wqk_sb = []
        wv_sb = []
        for k in range(KC):
            xt = const.tile([128, N], BF16, name=f"xts{k}", tag=f"xts{k}")
            nc.sync.dma_start(xt[:], d["xT"][128 * k : 128 * (k + 1), :])
            xts.append(xt)
            wq = const.tile([128, 512], BF16, name=f"wqk{k}", tag=f"wqk{k}")
            nc.sync.dma_start(wq[:], d["wqk"][128 * k : 128 * (k + 1), :])
            wqk_sb.append(wq)
            wv = const.tile([128, GC], BF16, name=f"wv{k}", tag=f"wv{k}")
            nc.sync.dma_start(wv[:], d["wv"][128 * k : 128 * (k + 1), :])
            wv_sb.append(wv)
        wo0_sb = const.tile([128, D], BF16, name="wo0", tag="wo0")
        nc.sync.dma_start(wo0_sb[:], d["wo0"][:, :])
        wo1_sb = const.tile([64, D], BF16, name="wo1", tag="wo1")
        nc.sync.dma_start(wo1_sb[:], d["wo1"][:, :])
        bqk_sb = const.tile([128, 4], F32, name="bqk", tag="bqk")
        nc.sync.dma_start(bqk_sb[:], d["bqk"][:, :])
        masks_sb = const.tile([128, 384], BF16, name="masks", tag="masks")
        nc.sync.dma_start(masks_sb[:], d["masks"][:, :])
        ident_sb = const.tile([128, 128], BF16, name="ident", tag="ident")
        nc.sync.dma_start(ident_sb[:], d["ident"][:, :])

        # ---- persistent intermediates ----
        # packed [q0|q1], [q2|pad], [k0|k1], [k2|pad] d-major, 4 chunks of
        # 128 rows x N cols (padding keeps q_h and k_h at the same
        # partition offset — matmul requires matching base partitions)
        qk_sb = [
            persist.tile([128, N], BF16, name=f"qk{m}", tag=f"qk{m}") for m in range(4)
        ]
        # v token-major with a ones column per head: per 128-token chunk the
        # 195 cols are [v_h0(64) | 1 | v_h1(64) | 1 | v_h2(64) | 1]
        v3a = persist.tile([128, NB * 195], BF16, name="v3a", tag="v3a")
        ones_ap = v3a[:].rearrange("p (c g s) -> p c g s", c=NB, g=HPC, s=65)[
            :, :, :, 64:65
        ]
        nc.vector.memset(ones_ap, 1.0)
        # attention output, head-major (A0: packed rows 0..127, A1: 128..191)
        a0_sb = persist.tile([128, N], BF16, name="a0", tag="a0")
        a1_sb = persist.tile([64, N], BF16, name="a1", tag="a1")

        # ---- q/k projections: qk^T[m] = wqk[:, m-chunk]^T @ x^T ----
        for m in range(4):
            for n in range(N // 512):
                ps = ps_big.tile([128, 512], F32, name=f"psqk{m}_{n}", tag="big")
                for k in range(KC):
                    nc.tensor.matmul(
                        ps[:],
                        wqk_sb[k][:, 128 * m : 128 * (m + 1)],
                        xts[k][:, 512 * n : 512 * (n + 1)],
                        start=(k == 0),
                        stop=(k == KC - 1),
                    )
                nc.scalar.activation(
                    qk_sb[m][:, 512 * n : 512 * (n + 1)],
                    ps[:],
                    AF.Identity,
                    bias=bqk_sb[:, m : m + 1],
                    scale=1.0,
                )

        # ---- v projection (token-major): v[i-chunk] = x^T[:, chunk]^T @ wv ----
        for i in range(NB):
            psv = ps_big.tile([128, GC], F32, name=f"psv{i}", tag="big")
            for k in range(KC):
                nc.tensor.matmul(
                    psv[:],
                    xts[k][:, 128 * i : 128 * (i + 1)],
                    wv_sb[k][:],
                    start=(k == 0),
                    stop=(k == KC - 1),
                )
            vout = v3a[:, 195 * i : 195 * i + 195].rearrange(
                "p (g s) -> p g s", g=HPC, s=65
            )[:, :, 0:64]
            vin = psv[:].rearrange("p (g s) -> p g s", g=HPC, s=DH)
            nc.vector.tensor_copy(vout, vin)

        # ---- banded attention, software-pipelined ----
        # PE stalls on the exp/mask chain if attnV(i) directly follows
        # scores(i): emit scores for block i but attnV/normalize/transpose
        # for block i-PIPE so the PE always has independent matmuls while
        # ACT/DVE fill E. Out-proj quarters are interleaved as soon as
        # their four attention blocks are flushed, keeping PE dense (HAM
        # re-throttles the PE clock after ~3.4us of idle).
        PIPE = 2
        stage = {}  # i -> (ao3, [(h, e, lo_c, hi_c)])

        def emit_scores(i):
            c_first = 1 if i == 0 else 0
            c_last = 1 if i == NB - 1 else 2
            lo, hi = 128 * c_first, 128 * (c_last + 1)
            ao3 = ao_pool.tile([128, GC], BF16, name=f"ao{i}", tag="ao")
            heads = []
            for h in range(HPC):
                # S^T[tk, tq] for key blocks j = i-1+c
                pss = ps_s.tile([128, 384], F32, name=f"pss{i}_{h}", tag="s")
                mq, qo = (0, 64 * h) if h < 2 else (1, 0)
                mk, ko = (2, 64 * h) if h < 2 else (3, 0)
                for c in range(c_first, c_last + 1):
                    j = i - 1 + c
                    nc.tensor.matmul(
                        pss[:, 128 * c : 128 * (c + 1)],
                        qk_sb[mk][ko : ko + 64, 128 * j : 128 * (j + 1)],
                        qk_sb[mq][qo : qo + 64, 128 * i : 128 * (i + 1)],
                        start=True,
                        stop=True,
                    )
                e = e_pool.tile([128, 384], BF16, name=f"e{i}_{h}", tag="e")
                nc.scalar.activation(e[:, lo:hi], pss[:, lo:hi], AF.Exp, scale=SCALE)
                nc.vector.tensor_mul(e[:, lo:hi], e[:, lo:hi], masks_sb[:, lo:hi])
                heads.append((h, e))
            stage[i] = (ao3, heads, c_first, c_last)

        def emit_attnv(i):
            ao3, heads, c_first, c_last = stage.pop(i)
            for h, e in heads:
                # attn @ [v | 1]: accumulates out[tq, 0:64] and Z[tq] in col 64
                pso = ps_sm.tile([128, 65], F32, name=f"pso{i}_{h}", tag="sm")
                for c in range(c_first, c_last + 1):
                    j = i - 1 + c
                    nc.tensor.matmul(
                        pso[:],
                        e[:, 128 * c : 128 * (c + 1)],
                        v3a[:, 195 * j + 65 * h : 195 * j + 65 * h + 65],
                        start=(c == c_first),
                        stop=(c == c_last),
                    )
                zr = zr_pool.tile([128, 1], F32, name=f"zr{i}_{h}", tag="zr")
                nc.vector.reciprocal(zr[:], pso[:, 64:65])
                nc.vector.tensor_scalar_mul(
                    ao3[:, 64 * h : 64 * (h + 1)], pso[:, 0:64], zr[:]
                )
            # transpose [tq, 192] -> head-major [192, tq] for the out-proj
            pt0 = ps_sm.tile([128, 128], BF16, name=f"pt0_{i}", tag="sm")
            nc.tensor.transpose(pt0[:], ao3[:, 0:128], ident_sb[:])
            pt1 = ps_sm.tile([64, 128], BF16, name=f"pt1_{i}", tag="sm")
            nc.tensor.transpose(pt1[:], ao3[:, 128:192], ident_sb[:])
            nc.vector.tensor_copy(a0_sb[:, 128 * i : 128 * (i + 1)], pt0[:])
            nc.vector.tensor_copy(a1_sb[:, 128 * i : 128 * (i + 1)], pt1[:])

        def emit_outproj(n):
            # partial out-projection for tokens [512n, 512(n+1))
            for dd in range(KC):
                psp = ps_big.tile([128, 512], F32, name=f"psp{n}_{dd}", tag="big")
                nc.tensor.matmul(
                    psp[:],
                    wo0_sb[:, 128 * dd : 128 * (dd + 1)],
                    a0_sb[:, 512 * n : 512 * (n + 1)],
                    start=True,
                    stop=False,
                )
                nc.tensor.matmul(
                    psp[:],
                    wo1_sb[:, 128 * dd : 128 * (dd + 1)],
                    a1_sb[:, 512 * n : 512 * (n + 1)],
                    start=False,
                    stop=True,
                )
                osb = o_pool.tile([128, 512], F32, name=f"os{n}_{dd}", tag="o")
                nc.scalar.copy(osb[:], psp[:])
                nc.sync.dma_start(
                    d["outT"][128 * dd : 128 * (dd + 1), 512 * n : 512 * (n + 1)],
                    osb[:],
                )

        flushed = 0
        for i in range(NB):
            emit_scores(i)
            if i >= PIPE:
                emit_attnv(i - PIPE)
                flushed = i - PIPE
                if (flushed + 1) % 4 == 0:
                    emit_outproj((flushed + 1) // 4 - 1)
        for i in range(NB - PIPE, NB):
            emit_attnv(i)
            if (i + 1) % 4 == 0:
                emit_outproj((i + 1) // 4 - 1)


_CACHED_NC = None


def build_nc():
    global _CACHED_NC
    if _CACHED_NC is not None:
        return _CACHED_NC
    nc = bass.Bass("TRN2", target_bir_lowering=False, debug=False, num_devices=NCORES)
    d = {
        "xT": nc.dram_tensor("xT", [D, N], BF16, kind="ExternalInput").ap(),
        "wqk": nc.dram_tensor("wqk", [D, 512], BF16, kind="ExternalInput").ap(),
        "wv": nc.dram_tensor("wv", [D, GC], BF16, kind="ExternalInput").ap(),
        "wo0": nc.dram_tensor("wo0", [128, D], BF16, kind="ExternalInput").ap(),
        "wo1": nc.dram_tensor("wo1", [64, D], BF16, kind="ExternalInput").ap(),
        "bqk": nc.dram_tensor("bqk", [128, 4], F32, kind="ExternalInput").ap(),
        "masks": nc.dram_tensor("masks", [128, 384], BF16, kind="ExternalInput").ap(),
        "ident": nc.dram_tensor("ident", [128, 128], BF16, kind="ExternalInput").ap(),
        "outT": nc.dram_tensor("outT", [D, N], F32, kind="ExternalOutput").ap(),
    }
    with tile.TileContext(nc) as tc:
        _emit(nc, tc, d)
    _split_multiwaits(nc)
    _CACHED_NC = nc
    return nc


def _build_masks():
    p = np.arange(128)[:, None]
    f = np.arange(128)[None, :]
    m = np.zeros((128, 384), np.float32)
    m[:, 0:128] = (p - f >= 64).astype(np.float32)    # key block i-1
    m[:, 128:256] = (np.abs(p - f) <= 64).astype(np.float32)  # key block i
    m[:, 256:384] = (p - f <= -64).astype(np.float32)  # key block i+1
    return m.astype(ml_dtypes.bfloat16)


def make_in_maps(x, Wq, bq, Wk, bk, Wv, bv, Wo, bo):
    bf = ml_dtypes.bfloat16
    masks = _build_masks()
    ident = np.eye(128, dtype=bf)
    xT = [np.ascontiguousarray(x[b].T).astype(bf) for b in range(B)]
    in_maps = []
    pad64 = np.zeros((D, 64), np.float32)
    padb = np.zeros(64, np.float32)
    for c in range(NCORES):
        b, g = divmod(c, 4)
        s = slice(GC * g, GC * (g + 1))
        # chunks: [q0|q1], [q2|pad], [k0|k1], [k2|pad]
        wqk = np.concatenate(
            [Wq[:, s][:, :128], Wq[:, s][:, 128:], pad64,
             Wk[:, s][:, :128], Wk[:, s][:, 128:], pad64],
            axis=1,
        ).astype(bf)
        wv = np.ascontiguousarray(Wv[:, s]).astype(bf)
        wo = Wo[s, :]
        bqk = np.concatenate(
            [bq[s][:128], bq[s][128:], padb, bk[s][:128], bk[s][128:], padb]
        ).reshape(4, 128).T
        in_maps.append(
            {
                "xT": xT[b],
                "wqk": wqk,
                "wv": wv,
                "wo0": np.ascontiguousarray(wo[0:128, :]).astype(bf),
                "wo1": np.ascontiguousarray(wo[128:GC, :]).astype(bf),
                "bqk": np.ascontiguousarray(bqk, dtype=np.float32),
                "masks": masks,
                "ident": ident,
            }
        )
    return in_maps


def combine_outputs(partials, Wq, bq, Wk, bk, Wv, bv, Wo, bo):
    const = (bv.astype(np.float32) @ Wo.astype(np.float32) + bo).astype(np.float32)
    out = np.empty((B, N, D), np.float32)
    for b in range(B):
        acc = partials[4 * b].astype(np.float32).copy()
        for c in range(4 * b + 1, 4 * b + 4):
            acc += partials[c]
        out[b] = acc.T + const
    return out


def kernel(x, Wq, bq, Wk, bk, Wv, bv, Wo, bo, _trace=False, **run_kwargs):
    x = np.asarray(x, dtype=np.float32)
    args = [np.asarray(a, dtype=np.float32) for a in (Wq, bq, Wk, bk, Wv, bv, Wo, bo)]
    nc = build_nc()
    in_maps = make_in_maps(x, *args)
    res = run_bass_kernel_spmd(
        nc, in_maps, core_ids=list(range(NCORES)), trace=_trace, **run_kwargs
    )
    partials = [res.results[c]["outT"] for c in range(NCORES)]
    out = combine_outputs(partials, *args)
    if _trace:
        kernel.last_results = res
    return out
